# revision 5
# baseline (speedup 1.0000x reference)
"""Trainium2 Bass kernel for the HCFDA dense-CNN module.

Math used (exact reassociations of the reference):
  1. The 256x256 1x1 DCT conv is only consumed through a channel-mean, so
     temp[b,h,w] = sum_c m[c] * x[b,c,h,w]  with  m = dct_w.mean(axis=0).
  2. Each diffusion step's 3x3 reflect-pad conv has equal (and symmetric)
     top/bottom kernel rows, so with A = shiftW_l(T)+shiftW_r(T) and
     Ca_raw = A + (b/a)*T it collapses to
       T' = c2*T + G @ Ca_raw + c1*Ca_raw
     where G = (alpha*a*(S_up+S_dn)).T is a 128x128 reflect-shift matrix.
     (The Laplacian is transpose-symmetric, so this also works on temp^T.)
  3. SE branch: pooled stats -> two tiny FCs -> sigmoid, per reference.
  out = x * sigmoid(att[c] * sigmoid(T3)[h,w])

Implementation notes:
  - temp is built transposed ([w,h]) by x-stationary N=1 matmuls so it lands
    directly as a 2D PSUM image; PE-transpose restores [h,w] for 'heat'.
  - att (x) heat broadcast is done with K=1 outer-product matmuls on the
    otherwise idle PE in phase B (fp32 GEMMs are ~3-pass; K=1/N=512 is cheap).
  - engine balance: sum-pool on ACT (accum_out), max-pool on DVE,
    phase-B mults split between DVE (reads PSUM) and GpSimd (via SBUF).

Sharding: pure data parallel, one batch element per NeuronCore (B=8).
"""

import numpy as np
from contextlib import ExitStack

B, C, H, W = 8, 256, 128, 128
HW = H * W          # 16384
NCHUNK = 8          # DMA/compute chunks over HW
CH = HW // NCHUNK   # 2048 columns per chunk
N_CORES = 8


def _reflect(i, n):
    if i < 0:
        return -i
    if i >= n:
        return 2 * (n - 1) - i
    return i


def _build_program(ratio, c1, c2):
    from concourse import bass, mybir, tile
    from concourse import bacc

    f32 = mybir.dt.float32
    AF = mybir.ActivationFunctionType
    ALU = mybir.AluOpType
    AX = mybir.AxisListType

    nc = bacc.Bacc("TRN2", target_bir_lowering=False, debug=False,
                   num_devices=N_CORES)

    xb = nc.dram_tensor("xb", [C, HW], f32, kind="ExternalInput").ap()
    mv = nc.dram_tensor("mv", [128, 2], f32, kind="ExternalInput").ap()
    gm = nc.dram_tensor("gm", [128, 128], f32, kind="ExternalInput").ap()
    idm = nc.dram_tensor("idm", [128, 128], f32, kind="ExternalInput").ap()
    w1d = nc.dram_tensor("w1t", [128, 32], f32, kind="ExternalInput").ap()
    w2d = nc.dram_tensor("w2t", [16, 256], f32, kind="ExternalInput").ap()
    b1d = nc.dram_tensor("b1c", [16, 1], f32, kind="ExternalInput").ap()
    b2d = nc.dram_tensor("b2r", [1, 256], f32, kind="ExternalInput").ap()
    outd = nc.dram_tensor("out", [C, HW], f32, kind="ExternalOutput").ap()

    with tile.TileContext(nc) as tc, ExitStack() as ctx:
        const = ctx.enter_context(tc.tile_pool(name="const", bufs=1))
        xpool = ctx.enter_context(tc.tile_pool(name="xp", bufs=1))
        work = ctx.enter_context(tc.tile_pool(name="work", bufs=2))
        stat = ctx.enter_context(tc.tile_pool(name="stat", bufs=1))

        m_sb = const.tile([128, 2], f32, tag="m", name="m")
        nc.sync.dma_start(out=m_sb[:], in_=mv)
        g_sb = const.tile([128, 128], f32, tag="g", name="g")
        nc.sync.dma_start(out=g_sb[:], in_=gm)
        id_sb = const.tile([128, 128], f32, tag="idm", name="idm")
        nc.sync.dma_start(out=id_sb[:], in_=idm)
        w1_sb = const.tile([128, 32], f32, tag="w1", name="w1")
        nc.sync.dma_start(out=w1_sb[:], in_=w1d)
        w2_sb = const.tile([16, 256], f32, tag="w2", name="w2")
        nc.sync.dma_start(out=w2_sb[:], in_=w2d)
        b1_sb = const.tile([16, 1], f32, tag="b1", name="b1")
        nc.sync.dma_start(out=b1_sb[:], in_=b1d)
        b2_sb = const.tile([1, 256], f32, tag="b2", name="b2")
        nc.sync.dma_start(out=b2_sb[:], in_=b2d)

        sums = stat.tile([128, 2, NCHUNK], f32, tag="sums", name="sums")
        maxs = stat.tile([128, 2, NCHUNK], f32, tag="maxs", name="maxs")
        Tp = [stat.tile([128, W + 2], f32, tag=f"Tp{i}", name=f"Tp{i}")
              for i in range(4)]
        junk = stat.tile([128, CH], f32, tag="junk", name="junk")
        heat = stat.tile([128, W], f32, tag="heat", name="heat")
        attT = [stat.tile([1, 128], f32, tag=f"attT{t}", name=f"attT{t}")
                for t in range(2)]

        xt = {}
        with ExitStack() as actx:
            psT = actx.enter_context(
                tc.tile_pool(name="psT", bufs=1, space="PSUM"))
            psD = actx.enter_context(
                tc.tile_pool(name="psD", bufs=1, space="PSUM"))
            psF = actx.enter_context(
                tc.tile_pool(name="psF", bufs=2, space="PSUM"))

            # ---------- Phase A: load x; x-stationary GEMV -> temp^T ----
            # pstemp[w, h] = sum_c m[c] * x[c, h*128+w]
            pstemp = psT.tile([128, 128], f32, tag="pstemp", name="pstemp")
            for j in range(NCHUNK):
                for t in range(2):
                    xt[t, j] = xpool.tile([128, CH], f32, tag=f"x{t}_{j}",
                                          name=f"x{t}_{j}")
                    nc.sync.dma_start(
                        out=xt[t, j][:],
                        in_=xb[t * 128:(t + 1) * 128, j * CH:(j + 1) * CH])
                for q in range(16):
                    h = 16 * j + q  # image row h == hw chunk of 128
                    nc.tensor.matmul(pstemp[:, h:h + 1],
                                     xt[0, j][:, q * 128:(q + 1) * 128],
                                     m_sb[:, 0:1], start=True, stop=False)
                    nc.tensor.matmul(pstemp[:, h:h + 1],
                                     xt[1, j][:, q * 128:(q + 1) * 128],
                                     m_sb[:, 1:2], start=False, stop=True)
                for t in range(2):
                    nc.scalar.activation(junk[:], xt[t, j][:], AF.Copy,
                                         accum_out=sums[:, t, j:j + 1])
                    nc.vector.reduce_max(maxs[:, t, j:j + 1], xt[t, j][:],
                                         axis=AX.X)

            # ---------- pooled stats finalize ----------
            ymax = stat.tile([128, 2], f32, tag="ymax", name="ymax")
            yavg = stat.tile([128, 2], f32, tag="yavg", name="yavg")
            ysum = stat.tile([128, 2], f32, tag="ysum", name="ysum")
            for t in range(2):
                nc.vector.reduce_sum(ysum[:, t:t + 1], sums[:, t, :],
                                     axis=AX.X)
                nc.vector.reduce_max(ymax[:, t:t + 1], maxs[:, t, :],
                                     axis=AX.X)
            nc.vector.tensor_scalar_mul(yavg[:], ysum[:], 1.0 / HW)

            # ---------- diffusion (on temp^T; kernel is symmetric) ------
            nc.vector.tensor_copy(Tp[0][:, 1:W + 1], pstemp[:])
            nc.vector.tensor_copy(Tp[0][:, 0:1], Tp[0][:, 2:3])
            nc.vector.tensor_copy(Tp[0][:, W + 1:W + 2], Tp[0][:, W - 1:W])
            for i in range(3):
                cur, nxt = Tp[i], Tp[i + 1]
                A = work.tile([128, W], f32, tag="dA", name="dA")
                nc.vector.tensor_add(A[:], cur[:, 0:W], cur[:, 2:W + 2])
                Ca = work.tile([128, W], f32, tag="dCa", name="dCa")
                nc.vector.scalar_tensor_tensor(Ca[:], cur[:, 1:W + 1],
                                               float(ratio), A[:],
                                               op0=ALU.mult, op1=ALU.add)
                pd = psD.tile([128, W], f32, tag="psD", name="psD")
                nc.tensor.matmul(pd[:], g_sb[:], Ca[:], start=True, stop=True)
                U = work.tile([128, W], f32, tag="dU", name="dU")
                nc.vector.scalar_tensor_tensor(U[:], Ca[:], float(c1), pd[:],
                                               op0=ALU.mult, op1=ALU.add)
                nc.vector.scalar_tensor_tensor(nxt[:, 1:W + 1],
                                               cur[:, 1:W + 1], float(c2),
                                               U[:], op0=ALU.mult, op1=ALU.add)
                nc.vector.tensor_copy(nxt[:, 0:1], nxt[:, 2:3])
                nc.vector.tensor_copy(nxt[:, W + 1:W + 2], nxt[:, W - 1:W])

            # transpose temp3 back to [h,w]; sigmoid folded into PSUM read
            pt = psD.tile([128, W], f32, tag="psD", name="ptr")
            nc.tensor.transpose(pt[:], Tp[3][:, 1:W + 1], id_sb[:])
            nc.scalar.activation(heat[:], pt[:], AF.Sigmoid)

            # ---------- SE FC chain (row-oriented FC2 -> attT) ----------
            rows = {}
            for bname, yv in (("avg", yavg), ("max", ymax)):
                ph = psF.tile([16, 1], f32, tag="psF", name=f"ph_{bname}")
                nc.tensor.matmul(ph[:], w1_sb[:, 0:16], yv[:, 0:1],
                                 start=True, stop=False)
                nc.tensor.matmul(ph[:], w1_sb[:, 16:32], yv[:, 1:2],
                                 start=False, stop=True)
                hb = stat.tile([16, 1], f32, tag=f"h_{bname}",
                               name=f"h_{bname}")
                nc.scalar.activation(hb[:], ph[:], AF.Relu, bias=b1_sb[:])
                for t in range(2):
                    pa = psF.tile([1, 128], f32, tag="psF",
                                  name=f"pa_{bname}{t}")
                    nc.tensor.matmul(pa[:], hb[:],
                                     w2_sb[:, t * 128:(t + 1) * 128],
                                     start=True, stop=True)
                    sg = stat.tile([1, 128], f32, tag=f"sg_{bname}{t}",
                                   name=f"sg_{bname}{t}")
                    nc.vector.tensor_add(sg[:], pa[:],
                                         b2_sb[0:1, t * 128:(t + 1) * 128])
                    nc.scalar.activation(sg[:], sg[:], AF.Sigmoid)
                    rows[bname, t] = sg
            for t in range(2):
                nc.vector.tensor_add(attT[t][:], rows["avg", t][:],
                                     rows["max", t][:])

        # ---------- Phase B: att (x) heat via PE, sigmoid, apply --------
        with tc.tile_pool(name="psB", bufs=2, space="PSUM") as psB:
            for j in range(NCHUNK):
                hrow = work.tile([1, CH], f32, tag="hrow", name="hrow")
                nc.sync.dma_start(out=hrow[:],
                                  in_=heat[16 * j:16 * j + 16, :])
                for t in range(2):
                    pb = psB.tile([128, CH], f32, tag="psB", name="psB")
                    for q in range(4):
                        nc.tensor.matmul(pb[:, q * 512:(q + 1) * 512],
                                         attT[t][:],
                                         hrow[0:1, q * 512:(q + 1) * 512],
                                         start=True, stop=True)
                    if t == 0:
                        nc.scalar.activation(pb[:], pb[:], AF.Sigmoid)
                        nc.vector.tensor_mul(xt[t, j][:], xt[t, j][:], pb[:])
                    else:
                        sc = work.tile([128, CH], f32, tag="sc", name="sc")
                        nc.scalar.activation(sc[:], pb[:], AF.Sigmoid)
                        nc.gpsimd.tensor_mul(xt[t, j][:], xt[t, j][:], sc[:])
                    nc.sync.dma_start(
                        out=outd[t * 128:(t + 1) * 128, j * CH:(j + 1) * CH],
                        in_=xt[t, j][:])

    nc.compile()
    return nc


_prog_cache = {}
_TRACE = False      # test harness sets True to collect an NTFF profile
_last_res = None    # BassKernelResults of the most recent run


def kernel(x, dct_w, w1, b1, w2, b2, alpha, lap):
    x = np.ascontiguousarray(np.asarray(x, dtype=np.float32))
    dct_w = np.asarray(dct_w, dtype=np.float32)
    w1 = np.asarray(w1, dtype=np.float32)
    b1 = np.asarray(b1, dtype=np.float32)
    w2 = np.asarray(w2, dtype=np.float32)
    b2 = np.asarray(b2, dtype=np.float32)
    alpha = float(np.asarray(alpha))
    lap = np.asarray(lap, dtype=np.float64)

    # decomposition requires the kernel's row structure (holds for HCFDA's
    # fixed Laplacian); verify.
    assert np.allclose(lap[0], lap[2]) and np.allclose(lap[:, 0], lap[:, 2])
    a, b = float(lap[0, 0]), float(lap[0, 1])
    ratio = b / a
    c1 = alpha * float(lap[1, 0])
    c2 = 1.0 + alpha * (float(lap[1, 1]) - float(lap[1, 0]) * b / a)

    m = dct_w.astype(np.float64).mean(axis=0)           # [C]
    S = np.zeros((H, H), dtype=np.float64)
    for h in range(H):
        S[h, _reflect(h - 1, H)] += 1.0
        S[h, _reflect(h + 1, H)] += 1.0
    G = (alpha * a) * S                                  # applied as G @ Ca_raw
    g_lhsT = np.ascontiguousarray(G.T.astype(np.float32))

    mv = np.ascontiguousarray(m.astype(np.float32).reshape(2, 128).T)  # [128,2]
    w1t = np.ascontiguousarray(
        w1.T.reshape(2, 128, 16).transpose(1, 0, 2).reshape(128, 32))
    w2t = np.ascontiguousarray(w2.T)                     # [16,256]
    b1c = np.ascontiguousarray(b1.reshape(16, 1))
    b2r = np.ascontiguousarray(b2.reshape(1, 256))
    idm = np.eye(128, dtype=np.float32)

    key = (ratio, c1, c2)
    if key not in _prog_cache:
        _prog_cache[key] = _build_program(ratio, c1, c2)
    nc = _prog_cache[key]

    consts = {"mv": mv, "gm": g_lhsT, "idm": idm, "w1t": w1t, "w2t": w2t,
              "b1c": b1c, "b2r": b2r}
    in_maps = [{"xb": np.ascontiguousarray(x[i].reshape(C, HW)), **consts}
               for i in range(N_CORES)]

    from concourse.bass_utils import run_bass_kernel_spmd
    res = run_bass_kernel_spmd(nc, in_maps, list(range(N_CORES)),
                               trace=_TRACE)
    global _last_res
    _last_res = res
    out = np.stack([res.results[i]["out"].reshape(C, H, W)
                    for i in range(N_CORES)])
    return out.astype(np.float32)


# revision 9
# speedup vs baseline: 1.5184x; 1.5184x over previous
"""Trainium2 Bass kernel for the HCFDA dense-CNN module.

Math used (exact reassociations of the reference):
  1. The 256x256 1x1 DCT conv is only consumed through a channel-mean, so
     temp[b,h,w] = sum_c m[c] * x[b,c,h,w]  with  m = dct_w.mean(axis=0).
  2. Each diffusion step's 3x3 reflect-pad conv has equal (and symmetric)
     top/bottom kernel rows, so with A = shiftW_l(T)+shiftW_r(T) and
     Ca_raw = A + (b/a)*T it collapses to
       T' = c2*T + G @ Ca_raw + c1*Ca_raw
     where G = (alpha*a*(S_up+S_dn)).T is a 128x128 reflect-shift matrix.
  3. SE branch: pooled stats -> two tiny FCs -> sigmoid, per reference.
  out = x * sigmoid(att[c] * sigmoid(T3)[h,w])

Implementation notes:
  - temp GEMV runs m-stationary with N=512 moving tiles in float32r
    (1 cycle/row vs fp32's 4) on the PE; plain fp32 everywhere else.
  - heat broadcast across channel partitions via gpsimd.partition_broadcast;
    sigmoid(att*heat) fused on ScalarE via per-partition scale.
  - engine balance: sum-pool on ACT (accum_out), max-pool + final mult on
    DVE, broadcast on GpSimd, GEMV + diffusion shifts + tiny FCs on PE.

Sharding: pure data parallel, one batch element per NeuronCore (B=8).
"""

import numpy as np
from contextlib import ExitStack

B, C, H, W = 8, 256, 128, 128
HW = H * W           # 16384
NCHUNK = 8           # x DMA chunks over HW
CH = HW // NCHUNK    # 2048
NB = 16              # phase-B chunks
CB = HW // NB        # 1024
N_CORES = 8


def _reflect(i, n):
    if i < 0:
        return -i
    if i >= n:
        return 2 * (n - 1) - i
    return i


def _build_program(ratio, c1, c2):
    from concourse import bass, mybir, tile
    from concourse import bacc

    f32 = mybir.dt.float32
    f32r = mybir.dt.float32r
    AF = mybir.ActivationFunctionType
    ALU = mybir.AluOpType
    AX = mybir.AxisListType

    nc = bacc.Bacc("TRN2", target_bir_lowering=False, debug=False,
                   num_devices=N_CORES)

    xb = nc.dram_tensor("xb", [C, HW], f32r, kind="ExternalInput").ap()
    mv = nc.dram_tensor("mv", [128, 2], f32r, kind="ExternalInput").ap()
    gm = nc.dram_tensor("gm", [128, 128], f32, kind="ExternalInput").ap()
    w1d = nc.dram_tensor("w1t", [128, 32], f32, kind="ExternalInput").ap()
    w2d = nc.dram_tensor("w2t", [16, 256], f32, kind="ExternalInput").ap()
    b1d = nc.dram_tensor("b1c", [16, 1], f32, kind="ExternalInput").ap()
    b2d = nc.dram_tensor("b2c", [128, 2], f32, kind="ExternalInput").ap()
    outd = nc.dram_tensor("out", [C, HW], f32, kind="ExternalOutput").ap()

    with tile.TileContext(nc) as tc, ExitStack() as ctx:
        const = ctx.enter_context(tc.tile_pool(name="const", bufs=1))
        xpool = ctx.enter_context(tc.tile_pool(name="xp", bufs=1))
        work = ctx.enter_context(tc.tile_pool(name="work", bufs=2))
        stat = ctx.enter_context(tc.tile_pool(name="stat", bufs=1))
        psA = ctx.enter_context(tc.tile_pool(name="psA", bufs=2, space="PSUM"))
        psD = ctx.enter_context(tc.tile_pool(name="psD", bufs=1, space="PSUM"))
        psF = ctx.enter_context(tc.tile_pool(name="psF", bufs=2, space="PSUM"))

        m_sb = const.tile([128, 2], f32r, tag="m", name="m")
        nc.sync.dma_start(out=m_sb[:], in_=mv)
        g_sb = const.tile([128, 128], f32, tag="g", name="g")
        nc.sync.dma_start(out=g_sb[:], in_=gm)
        w1_sb = const.tile([128, 32], f32, tag="w1", name="w1")
        nc.sync.dma_start(out=w1_sb[:], in_=w1d)
        w2_sb = const.tile([16, 256], f32, tag="w2", name="w2")
        nc.sync.dma_start(out=w2_sb[:], in_=w2d)
        b1_sb = const.tile([16, 1], f32, tag="b1", name="b1")
        nc.sync.dma_start(out=b1_sb[:], in_=b1d)
        b2_sb = const.tile([128, 2], f32, tag="b2", name="b2")
        nc.sync.dma_start(out=b2_sb[:], in_=b2d)

        sums = stat.tile([128, 2, NCHUNK], f32, tag="sums", name="sums")
        maxs = stat.tile([128, 2, NCHUNK], f32, tag="maxs", name="maxs")
        Tp = [stat.tile([128, W + 2], f32, tag=f"Tp{i}", name=f"Tp{i}")
              for i in range(4)]
        junk = stat.tile([128, CH], f32, tag="junk", name="junk")
        heat = stat.tile([128, W], f32, tag="heat", name="heat")

        # ---------- Phase A: load x; GEMV temp; pooled stats ----------
        xt = {}
        for j in range(NCHUNK):
            for t in range(2):
                xt[t, j] = xpool.tile([128, CH], f32r, tag=f"x{t}_{j}",
                                      name=f"x{t}_{j}")
                nc.sync.dma_start(
                    out=xt[t, j][:],
                    in_=xb[t * 128:(t + 1) * 128, j * CH:(j + 1) * CH])
            for half in range(2):
                k = 2 * j + half  # 1024-col temp chunk -> rows 8k..8k+7
                ps = psA.tile([1, 1024], f32, tag="psA", name="psA")
                for s in range(2):
                    col = half * 1024 + s * 512
                    nc.tensor.matmul(
                        ps[:, s * 512:(s + 1) * 512],
                        m_sb[:, 0:1],
                        xt[0, j][:, col:col + 512],
                        start=True, stop=False)
                    nc.tensor.matmul(
                        ps[:, s * 512:(s + 1) * 512],
                        m_sb[:, 1:2],
                        xt[1, j][:, col:col + 512],
                        start=False, stop=True)
                trow = work.tile([1, 1024], f32, tag="trow", name="trow")
                nc.scalar.copy(trow[:], ps[:])
                nc.sync.dma_start(out=Tp[0][8 * k:8 * k + 8, 1:W + 1],
                                  in_=trow[:])
            for t in range(2):
                xf = xt[t, j][:].bitcast(f32)
                nc.scalar.activation(junk[:], xf, AF.Copy,
                                     accum_out=sums[:, t, j:j + 1])
                nc.vector.reduce_max(maxs[:, t, j:j + 1], xf, axis=AX.X)

        # ---------- pooled stats finalize ----------
        ymax = stat.tile([128, 2], f32, tag="ymax", name="ymax")
        yavg = stat.tile([128, 2], f32, tag="yavg", name="yavg")
        ysum = stat.tile([128, 2], f32, tag="ysum", name="ysum")
        for t in range(2):
            nc.vector.reduce_sum(ysum[:, t:t + 1], sums[:, t, :], axis=AX.X)
            nc.vector.reduce_max(ymax[:, t:t + 1], maxs[:, t, :], axis=AX.X)
        nc.vector.tensor_scalar_mul(yavg[:], ysum[:], 1.0 / HW)

        # ---------- diffusion: 3 steps ----------
        nc.vector.tensor_copy(Tp[0][:, 0:1], Tp[0][:, 2:3])
        nc.vector.tensor_copy(Tp[0][:, W + 1:W + 2], Tp[0][:, W - 1:W])
        for i in range(3):
            cur, nxt = Tp[i], Tp[i + 1]
            A = work.tile([128, W], f32, tag="dA", name="dA")
            nc.vector.tensor_add(A[:], cur[:, 0:W], cur[:, 2:W + 2])
            Ca = work.tile([128, W], f32, tag="dCa", name="dCa")
            nc.vector.scalar_tensor_tensor(Ca[:], cur[:, 1:W + 1],
                                           float(ratio), A[:],
                                           op0=ALU.mult, op1=ALU.add)
            pd = psD.tile([128, W], f32, tag="psD", name="psD")
            nc.tensor.matmul(pd[:], g_sb[:], Ca[:], start=True, stop=True)
            U = work.tile([128, W], f32, tag="dU", name="dU")
            nc.vector.scalar_tensor_tensor(U[:], Ca[:], float(c1), pd[:],
                                           op0=ALU.mult, op1=ALU.add)
            nc.vector.scalar_tensor_tensor(nxt[:, 1:W + 1], cur[:, 1:W + 1],
                                           float(c2), U[:],
                                           op0=ALU.mult, op1=ALU.add)
            nc.vector.tensor_copy(nxt[:, 0:1], nxt[:, 2:3])
            nc.vector.tensor_copy(nxt[:, W + 1:W + 2], nxt[:, W - 1:W])

        nc.scalar.activation(heat[:], Tp[3][:, 1:W + 1], AF.Sigmoid)

        # ---------- SE FC chain ----------
        att = stat.tile([128, 2], f32, tag="att", name="att")
        sgs = {}
        for bname, yv in (("avg", yavg), ("max", ymax)):
            ph = psF.tile([16, 1], f32, tag="psF", name=f"ph_{bname}")
            nc.tensor.matmul(ph[:], w1_sb[:, 0:16], yv[:, 0:1],
                             start=True, stop=False)
            nc.tensor.matmul(ph[:], w1_sb[:, 16:32], yv[:, 1:2],
                             start=False, stop=True)
            hb = stat.tile([16, 1], f32, tag=f"h_{bname}", name=f"h_{bname}")
            nc.scalar.activation(hb[:], ph[:], AF.Relu, bias=b1_sb[:])
            for t in range(2):
                pa = psF.tile([128, 1], f32, tag="psF", name=f"pa_{bname}{t}")
                nc.tensor.matmul(pa[:], w2_sb[:, t * 128:(t + 1) * 128],
                                 hb[:], start=True, stop=True)
                sg = stat.tile([128, 1], f32, tag=f"sg_{bname}{t}",
                               name=f"sg_{bname}{t}")
                nc.scalar.activation(sg[:], pa[:], AF.Sigmoid,
                                     bias=b2_sb[:, t:t + 1])
                sgs[bname, t] = sg
        for t in range(2):
            nc.vector.tensor_add(att[:, t:t + 1], sgs["avg", t][:],
                                 sgs["max", t][:])

        # ---------- Phase B: broadcast heat, apply attention, store -----
        for j in range(NB):
            hrow = work.tile([1, CB], f32, tag="hrow", name="hrow", bufs=2)
            nc.sync.dma_start(out=hrow[:], in_=heat[8 * j:8 * j + 8, :])
            hb2 = work.tile([128, CB], f32, tag="hb2", name="hb2", bufs=3)
            nc.gpsimd.partition_broadcast(hb2[:], hrow[:])
            jj, half = j // 2, j % 2
            for t in range(2):
                xs = xt[t, jj][:, half * CB:(half + 1) * CB].bitcast(f32)
                sc = work.tile([128, CB], f32, tag="sc", name="sc", bufs=3)
                nc.scalar.activation(sc[:], hb2[:], AF.Sigmoid,
                                     scale=att[:, t:t + 1])
                nc.vector.tensor_mul(sc[:], xs, sc[:])
                nc.sync.dma_start(
                    out=outd[t * 128:(t + 1) * 128, j * CB:(j + 1) * CB],
                    in_=sc[:])

    nc.compile()
    return nc


_prog_cache = {}
_TRACE = False      # test harness sets True to collect an NTFF profile
_last_res = None    # BassKernelResults of the most recent run


def kernel(x, dct_w, w1, b1, w2, b2, alpha, lap):
    x = np.ascontiguousarray(np.asarray(x, dtype=np.float32))
    dct_w = np.asarray(dct_w, dtype=np.float32)
    w1 = np.asarray(w1, dtype=np.float32)
    b1 = np.asarray(b1, dtype=np.float32)
    w2 = np.asarray(w2, dtype=np.float32)
    b2 = np.asarray(b2, dtype=np.float32)
    alpha = float(np.asarray(alpha))
    lap = np.asarray(lap, dtype=np.float64)

    # decomposition requires the kernel's row structure (holds for HCFDA's
    # fixed Laplacian); verify.
    assert np.allclose(lap[0], lap[2]) and np.allclose(lap[:, 0], lap[:, 2])
    a, b = float(lap[0, 0]), float(lap[0, 1])
    ratio = b / a
    c1 = alpha * float(lap[1, 0])
    c2 = 1.0 + alpha * (float(lap[1, 1]) - float(lap[1, 0]) * b / a)

    m = dct_w.astype(np.float64).mean(axis=0)           # [C]
    S = np.zeros((H, H), dtype=np.float64)
    for h in range(H):
        S[h, _reflect(h - 1, H)] += 1.0
        S[h, _reflect(h + 1, H)] += 1.0
    G = (alpha * a) * S                                  # applied as G @ Ca_raw
    g_lhsT = np.ascontiguousarray(G.T.astype(np.float32))

    mv = np.ascontiguousarray(m.astype(np.float32).reshape(2, 128).T)  # [128,2]
    w1t = np.ascontiguousarray(
        w1.T.reshape(2, 128, 16).transpose(1, 0, 2).reshape(128, 32))
    w2t = np.ascontiguousarray(w2.T)                     # [16,256]
    b1c = np.ascontiguousarray(b1.reshape(16, 1))
    b2c = np.ascontiguousarray(b2.reshape(2, 128).T)     # [128,2]

    key = (ratio, c1, c2)
    if key not in _prog_cache:
        _prog_cache[key] = _build_program(ratio, c1, c2)
    nc = _prog_cache[key]

    consts = {"mv": mv, "gm": g_lhsT, "w1t": w1t, "w2t": w2t,
              "b1c": b1c, "b2c": b2c}
    in_maps = [{"xb": np.ascontiguousarray(x[i].reshape(C, HW)), **consts}
               for i in range(N_CORES)]

    from concourse.bass_utils import run_bass_kernel_spmd
    res = run_bass_kernel_spmd(nc, in_maps, list(range(N_CORES)),
                               trace=_TRACE)
    global _last_res
    _last_res = res
    out = np.stack([res.results[i]["out"].reshape(C, H, W)
                    for i in range(N_CORES)])
    return out.astype(np.float32)
